# revision 1
# baseline (speedup 1.0000x reference)
"""Multi-head causal self-attention with RoPE on 8 Trainium2 NeuronCores.

Reference computation (B=2, S=2048, D=2048, H=16, DH=128):
    xs = hidden_q / sqrt(D)
    q,k,v = xs @ {Wq,Wk,Wv}.T        (reshaped to [B,H,S,DH])
    q,k <- RoPE(q,k)
    scores = q @ k.T / sqrt(DH)  (causal masked)
    p = softmax(scores); attn = p @ v
    out = (attn / sqrt(H*DH)) @ Wo.T

Sharding: 8 cores = 2 (batch) x 4 (head-groups of 4 heads).  Each core
computes its head-group's projections, attention and a partial output
projection; the host sums the 4 partials per batch.

All matmuls run in float32r (TF32-like, full PE rate at N=512).
Layouts on device (per core):
    xT   [D, S]    feature-major activations (host pre-transposed, pre-scaled)
    wqT  [D, 512]  per-group Wq slice, transposed
    scoresT [keys, queries] so softmax-denominators come from a ones-matmul
    attnT [dh, q] accumulated per head, normalized with broadcast reciprocal
    y    [S, D]    natural layout partial output (host sums over groups)
"""

import math
from contextlib import ExitStack

import numpy as np

import concourse.bass as bass
import concourse.mybir as mybir
import concourse.tile as tile
from concourse import bacc
from concourse.bass import ts
from concourse.bass_utils import run_bass_kernel_spmd
from concourse.masks import make_identity

B, S, D, H, DH = 2, 2048, 2048, 16, 128
BASE = 10000.0
G = 4              # head-groups (cores per batch)
HG = H // G        # heads per group = 4
F = HG * DH        # features per group = 512
NT = S // 128      # 16 token tiles
NQB = S // 512     # 4 query blocks
F32 = mybir.dt.float32
F32R = mybir.dt.float32r

_cache = {}


def _rope_tables():
    inv_freq = 1.0 / (BASE ** (np.arange(0, DH, 2, dtype=np.float64) / DH))
    t = np.arange(S, dtype=np.float64)
    freqs = np.outer(t, inv_freq)                       # [S, 64]
    return (np.cos(freqs).astype(np.float32), np.sin(freqs).astype(np.float32))


def _mask_tiles():
    # mask[o][j, q] = 1 if q >= j + 128*o else 0  (diagonal-band tiles)
    o = np.arange(4)[:, None, None]
    j = np.arange(128)[None, :, None]
    q = np.arange(512)[None, None, :]
    return (q >= j + 128 * o).astype(np.float32)        # [4, 128, 512]


def _build(reps=1):
    key = ("nc", reps)
    if key in _cache:
        return _cache[key]
    nc = bacc.Bacc("TRN2", target_bir_lowering=False, debug=False, num_devices=8)

    xT = nc.dram_tensor("xT", [D, S], F32R, kind="ExternalInput")
    wqT = nc.dram_tensor("wqT", [D, F], F32R, kind="ExternalInput")
    wkT = nc.dram_tensor("wkT", [D, F], F32R, kind="ExternalInput")
    wvT = nc.dram_tensor("wvT", [D, F], F32R, kind="ExternalInput")
    woT = nc.dram_tensor("woT", [F, D], F32R, kind="ExternalInput")
    cos_d = nc.dram_tensor("cos", [S, 64], F32R, kind="ExternalInput")
    sin_d = nc.dram_tensor("sin", [S, 64], F32R, kind="ExternalInput")
    msk_d = nc.dram_tensor("masks", [4, 128, 512], F32R, kind="ExternalInput")
    y = nc.dram_tensor("y", [S, D], F32, kind="ExternalOutput")

    # chunked spill tensors (one per 4-token-tile group) so phase-B reloads
    # depend only on their own chunk's spills, not the whole phase A
    q_spill = [nc.dram_tensor(f"q_spill{c}", [F, 512], F32R) for c in range(4)]
    k_spill = [nc.dram_tensor(f"k_spill{c}", [F, 512], F32R) for c in range(4)]
    q_spill_r = [t.ap().rearrange("(hb p) s -> p hb s", p=128) for t in q_spill]
    k_spill_r = [t.ap().rearrange("(hb p) s -> p hb s", p=128) for t in k_spill]

    xT_r = xT.ap().rearrange("(kt p) s -> p kt s", p=128)       # [128, 16, S]

    with tile.TileContext(nc) as tc, ExitStack() as ctx:
        const = ctx.enter_context(tc.tile_pool(name="const", bufs=1))
        vpool = ctx.enter_context(tc.tile_pool(name="vpool", bufs=1))
        ps512 = ctx.enter_context(tc.tile_pool(name="ps512", bufs=6, space="PSUM"))
        # transposes (phase A) and denominators (phase B) share slots
        ps_sm = ctx.enter_context(tc.tile_pool(name="ps_sm", bufs=2, space="PSUM"))

        ones_f = const.tile([128, 1], F32, tag="ones_f")
        nc.gpsimd.memset(ones_f[:], 1.0)
        ones = const.tile([128, 1], F32R, tag="ones")
        nc.vector.tensor_copy(ones[:], ones_f[:])
        ident_f = const.tile([128, 128], F32, tag="ident_f")
        make_identity(nc, ident_f[:])
        ident = const.tile([128, 128], F32R, tag="ident")
        nc.vector.tensor_copy(ident[:], ident_f[:])

        for _rep in range(reps):
            vh_cb = [vpool.tile([128, 4, F], F32R, tag=f"vh{c}", name=f"vh{c}") for c in range(4)]

            # ---------- Phase A: projections + RoPE + transpose + spill ----
            with ExitStack() as actx:
                wpool = actx.enter_context(tc.tile_pool(name="wpool", bufs=1))
                xpool = actx.enter_context(tc.tile_pool(name="xpool", bufs=3))
                rot_pool = actx.enter_context(tc.tile_pool(name="rot", bufs=2))
                tmp_pool = actx.enter_context(tc.tile_pool(name="tmp", bufs=4))
                stage = actx.enter_context(tc.tile_pool(name="stage", bufs=3))

                wq_sb = wpool.tile([128, NT, F], F32R, tag="wq")
                wk_sb = wpool.tile([128, NT, F], F32R, tag="wk")
                wv_sb = wpool.tile([128, NT, F], F32R, tag="wv")
                cos_sb = wpool.tile([128, NT, 64], F32R, tag="cos")
                sin_sb = wpool.tile([128, NT, 64], F32R, tag="sin")
                nc.sync.dma_start(cos_sb[:], cos_d.ap().rearrange("(t p) c -> p t c", p=128))
                nc.sync.dma_start(sin_sb[:], sin_d.ap().rearrange("(t p) c -> p t c", p=128))
                wqT_r = wqT.ap().rearrange("(kt p) f -> p kt f", p=128)
                wkT_r = wkT.ap().rearrange("(kt p) f -> p kt f", p=128)
                wvT_r = wvT.ap().rearrange("(kt p) f -> p kt f", p=128)
                # interleave x-tile prefetches into the weight stream so the
                # PE can chase the arriving weights through tb=0..2
                xq_tiles = {}
                for tb in range(3):
                    xq_tiles[tb] = xpool.tile([128, NT, 128], F32R, tag="xq", name=f"xq{tb}")
                nc.sync.dma_start(xq_tiles[0][:], xT_r[:, :, ts(0, 128)])
                for kt in range(NT):
                    nc.sync.dma_start(wq_sb[:, kt, :], wqT_r[:, kt, :])
                    nc.sync.dma_start(wk_sb[:, kt, :], wkT_r[:, kt, :])
                    nc.sync.dma_start(wv_sb[:, kt, :], wvT_r[:, kt, :])
                    if kt in (2, 5):
                        tb = 1 if kt == 2 else 2
                        nc.sync.dma_start(xq_tiles[tb][:], xT_r[:, :, ts(tb, 128)])

                for tb in range(NT):
                    if tb in xq_tiles:
                        xq = xq_tiles[tb]
                    else:
                        xq = xpool.tile([128, NT, 128], F32R, tag="xq")
                        nc.sync.dma_start(xq[:], xT_r[:, :, ts(tb, 128)])
                    pq = ps512.tile([128, 512], F32, tag="ps512")
                    pk = ps512.tile([128, 512], F32, tag="ps512")
                    pv = ps512.tile([128, 512], F32, tag="ps512")
                    for kt in range(NT):
                        f = dict(start=(kt == 0), stop=(kt == NT - 1))
                        nc.tensor.matmul(pq[:], xq[:, kt, :], wq_sb[:, kt, :], **f)
                        nc.tensor.matmul(pk[:], xq[:, kt, :], wk_sb[:, kt, :], **f)
                        nc.tensor.matmul(pv[:], xq[:, kt, :], wv_sb[:, kt, :], **f)
                    nc.vector.tensor_copy(vh_cb[tb // 4][:, tb % 4, :], pv[:])

                    # RoPE with broadcast APs: 4 wide DVE ops per tensor.
                    cos_b = cos_sb[:, tb, :].unsqueeze(1).unsqueeze(1) \
                        .broadcast_to((128, HG, 2, 64))
                    sin_b = sin_sb[:, tb, :].unsqueeze(1).broadcast_to((128, HG, 64))
                    for (ps, spill_r, rtag, stag) in (
                        (pq, q_spill_r, "qrot", "stq"),
                        (pk, k_spill_r, "krot", "stk"),
                    ):
                        ps_r = ps[:].rearrange("p (hb half j) -> p hb half j",
                                               hb=HG, half=2, j=64)
                        rot = rot_pool.tile([128, 512], F32R, tag=rtag)
                        rot_r = rot[:].rearrange("p (hb half j) -> p hb half j",
                                                 hb=HG, half=2, j=64)
                        tmp = tmp_pool.tile([128, HG, 2, 64], F32R, tag="tmp")
                        # tmp_lo = -q_hi * sin ; tmp_hi = +q_lo * sin
                        nc.vector.scalar_tensor_tensor(
                            tmp[:, :, 0, :], ps_r[:, :, 1, :], -1.0, sin_b,
                            op0=mybir.AluOpType.mult, op1=mybir.AluOpType.mult)
                        nc.vector.tensor_mul(tmp[:, :, 1, :], ps_r[:, :, 0, :], sin_b)
                        # rot = q * cos + tmp
                        nc.vector.tensor_mul(rot_r[:], ps_r[:], cos_b)
                        nc.vector.tensor_add(rot[:], rot[:],
                                             tmp[:].rearrange("p a b c -> p (a b c)"))
                        st = stage.tile([128, HG, 128], F32R, tag=stag)
                        for hb in range(HG):
                            ptr = ps_sm.tile([128, 128], F32R, tag="small")
                            nc.tensor.transpose(ptr[:], rot[:, ts(hb, 128)], ident[:])
                            nc.scalar.copy(st[:, hb, :], ptr[:])
                        nc.sync.dma_start(spill_r[tb // 4][:, :, ts(tb % 4, 128)], st[:])

            # ---------- Phase B+C: attention + output projection -----------
            with ExitStack() as bctx:
                mpool = bctx.enter_context(tc.tile_pool(name="mpool", bufs=1))
                pt_pool = bctx.enter_context(tc.tile_pool(name="pt", bufs=8))
                nrm = bctx.enter_context(tc.tile_pool(name="nrm", bufs=2))
                att_pool = bctx.enter_context(tc.tile_pool(name="attp", bufs=1))
                ystage = bctx.enter_context(tc.tile_pool(name="ystage", bufs=4))

                msk_sb = mpool.tile([128, 4, 512], F32R)
                nc.sync.dma_start(msk_sb[:], msk_d.ap().rearrange("o p q -> p o q"))
                wo_sb = mpool.tile([128, G, D], F32R, tag="wo")
                nc.sync.dma_start(wo_sb[:], woT.ap().rearrange("(ft p) d -> p ft d", p=128))
                qh_cb, kh_cb = [], []
                for cb in range(4):
                    qh = mpool.tile([128, HG, 512], F32R, tag=f"qh{cb}", name=f"qh{cb}")
                    kh = mpool.tile([128, HG, 512], F32R, tag=f"kh{cb}", name=f"kh{cb}")
                    for h in range(HG):
                        nc.gpsimd.dma_start(qh[:, h, :], q_spill_r[cb][:, h, :])
                        nc.gpsimd.dma_start(kh[:, h, :], k_spill_r[cb][:, h, :])
                    qh_cb.append(qh)
                    kh_cb.append(kh)
                attn_sb = att_pool.tile([128, HG, S], F32R, tag="attn_sb")

                for qb in range(NQB):
                    nkt = 4 * qb + 4
                    for h in range(HG):
                        p_att = ps512.tile([128, 512], F32, tag="ps512")
                        p_den = ps_sm.tile([1, 512], F32, tag="small")
                        for kt in range(nkt):
                            p_s = ps512.tile([128, 512], F32, tag="ps512")
                            nc.tensor.matmul(p_s[:],
                                             kh_cb[kt // 4][:, h, ts(kt % 4, 128)],
                                             qh_cb[qb][:, h, :],
                                             start=True, stop=True)
                            pt = pt_pool.tile([128, 512], F32R, tag="pt")
                            nc.scalar.activation(pt[:], p_s[:],
                                                 mybir.ActivationFunctionType.Exp,
                                                 scale=1.0 / math.sqrt(DH))
                            if kt >= 4 * qb:
                                nc.vector.tensor_mul(pt[:], pt[:],
                                                     msk_sb[:, kt - 4 * qb, :])
                            f = dict(start=(kt == 0), stop=(kt == nkt - 1))
                            nc.tensor.matmul(p_att[:],
                                             vh_cb[kt // 4][:, kt % 4, ts(h, 128)],
                                             pt[:], **f)
                            nc.tensor.matmul(p_den[:], ones[:], pt[:], **f)
                        recip = nrm.tile([1, 512], F32, tag="recip")
                        nc.vector.reciprocal_approx_fast(recip[:], p_den[:])
                        rb = nrm.tile([128, 512], F32, tag="rb")
                        nc.gpsimd.partition_broadcast(rb[:], recip[:])
                        nc.vector.tensor_mul(attn_sb[:, h, ts(qb, 512)],
                                             p_att[:], rb[:])
                    # output projection for this query block
                    for qt in range(4 * qb, 4 * qb + 4):
                        for ddb in range(NQB):
                            py = ps512.tile([128, 512], F32, tag="ps512")
                            for ft in range(G):
                                nc.tensor.matmul(py[:], attn_sb[:, ft, ts(qt, 128)],
                                                 wo_sb[:, ft, ts(ddb, 512)],
                                                 start=(ft == 0), stop=(ft == G - 1))
                            y_sb = ystage.tile([128, 512], F32, tag="ysb")
                            nc.scalar.copy(y_sb[:], py[:])
                            nc.sync.dma_start(y.ap()[ts(qt, 128), ts(ddb, 512)],
                                              y_sb[:])

    nc.compile()
    _cache[key] = nc
    return nc


def _in_maps(hidden_q, Wq, Wk, Wv, Wo):
    xs = (hidden_q.astype(np.float32) / math.sqrt(D))
    xT = [np.ascontiguousarray(xs[b].T) for b in range(B)]     # [D, S] each
    cos_t, sin_t = _rope_tables()
    masks = _mask_tiles()
    wo_s = Wo.astype(np.float32) / math.sqrt(H * DH)
    in_maps = []
    for c in range(8):
        b, g = c // G, c % G
        rows = slice(F * g, F * (g + 1))
        in_maps.append({
            "xT": xT[b],
            "wqT": np.ascontiguousarray(Wq[rows, :].T),
            "wkT": np.ascontiguousarray(Wk[rows, :].T),
            "wvT": np.ascontiguousarray(Wv[rows, :].T),
            "woT": np.ascontiguousarray(wo_s[:, rows].T),
            "cos": cos_t, "sin": sin_t, "masks": masks,
        })
    return in_maps


def kernel(hidden_q, attention_mask, position_bias, Wq, Wk, Wv, Wo):
    hidden_q = np.asarray(hidden_q)
    Wq, Wk, Wv, Wo = (np.asarray(w) for w in (Wq, Wk, Wv, Wo))
    assert hidden_q.shape == (B, S, D)
    in_maps = _in_maps(hidden_q, Wq, Wk, Wv, Wo)
    nc = _build()
    res = run_bass_kernel_spmd(nc, in_maps, core_ids=list(range(8)))
    _cache["last_results"] = res
    out = np.zeros((B, S, D), np.float32)
    for c in range(8):
        out[c // G] += res.results[c]["y"]
    return out



# revision 14
# speedup vs baseline: 1.0333x; 1.0333x over previous
"""Multi-head causal self-attention with RoPE on 8 Trainium2 NeuronCores.

Reference computation (B=2, S=2048, D=2048, H=16, DH=128):
    xs = hidden_q / sqrt(D)
    q,k,v = xs @ {Wq,Wk,Wv}.T        (reshaped to [B,H,S,DH])
    q,k <- RoPE(q,k)
    scores = q @ k.T / sqrt(DH)  (causal masked)
    p = softmax(scores); attn = p @ v
    out = (attn / sqrt(H*DH)) @ Wo.T

Sharding: 8 cores = 2 (batch) x 4 (head-groups of 4 heads).  Each core
computes its head-group's projections, attention and a partial output
projection; the host sums the 4 partials per batch.

v2 design (vs v1): all matmul operands in bf16 (PSUM accumulation stays
fp32); q/k are produced DIRECTLY TRANSPOSED by swapping matmul roles
(weights stationary, x moving -> out[feature, token]), which removes all
128 PE transposes, the 8MB q/k DRAM spill roundtrip, and the phase
barrier.  RoPE runs on DVE with partition-shifted APs (rotate_half swaps
partition halves in the transposed layout; the sin table carries the
sign).  Softmax denominators (ones-matmuls, M=1) are packed 4-at-a-time
onto disjoint PE column groups via tile_position so they run
concurrently (~4x less PE time than v1's serial den matmuls).

Layouts on device (per core):
    xT    [D, S]    feature-major activations (host pre-transposed/scaled)
    wqT   [D, 512]  per-group Wq slice, transposed (ditto wk, wv)
    woT   [512, D]  per-group Wo slice, transposed, prescaled
    qT/kT [128, h, S] SBUF: partition=dh-within-head, free=[head, token]
    vh    [128, tile, 512] SBUF: natural [token, feature] tiles
    scoresT [keys, queries]; denominators via packed ones-matmuls
    y     [S, D]    natural-layout partial output (host sums over groups)
"""

import math
from contextlib import ExitStack

import ml_dtypes
import numpy as np

import concourse.bass as bass
import concourse.mybir as mybir
import concourse.tile as tile
from concourse import bacc
from concourse.bass import ts
from concourse.bass_utils import run_bass_kernel_spmd

B, S, D, H, DH = 2, 2048, 2048, 16, 128
BASE = 10000.0
G = 4              # head-groups (cores per batch)
HG = H // G        # heads per group = 4
F = HG * DH        # features per group = 512
NT = S // 128      # 16 token tiles of 128
TB = S // 512      # 4 token blocks of 512
NQB = S // 512     # 4 query blocks
F32 = mybir.dt.float32
BF16 = mybir.dt.bfloat16
NPBF = ml_dtypes.bfloat16

_cache = {}


def _rope_tables():
    inv_freq = 1.0 / (BASE ** (np.arange(0, DH, 2, dtype=np.float64) / DH))
    t = np.arange(S, dtype=np.float64)
    freqs = np.outer(t, inv_freq)                       # [S, 64]
    cosT = np.cos(freqs).T.astype(np.float32)           # [64, S]
    sinT = np.sin(freqs).T.astype(np.float32)
    cos_full = np.ascontiguousarray(np.concatenate([cosT, cosT], 0)).astype(NPBF)
    sin_signed = np.ascontiguousarray(np.concatenate([-sinT, sinT], 0)).astype(NPBF)
    return cos_full, sin_signed                         # [128, S] each, bf16


def _mask_tiles():
    # mask[o][j, q] = 1 if q >= j + 128*o else 0  (diagonal-band tiles)
    o = np.arange(4)[:, None, None]
    j = np.arange(128)[None, :, None]
    q = np.arange(512)[None, None, :]
    return (q >= j + 128 * o).astype(NPBF)              # [4, 128, 512]


def _build(reps=1):
    key = ("nc", reps)
    if key in _cache:
        return _cache[key]
    nc = bacc.Bacc("TRN2", target_bir_lowering=False, debug=False, num_devices=8)

    xT = nc.dram_tensor("xT", [D, S], BF16, kind="ExternalInput")
    wqT = nc.dram_tensor("wqT", [D, F], BF16, kind="ExternalInput")
    wkT = nc.dram_tensor("wkT", [D, F], BF16, kind="ExternalInput")
    wvT = nc.dram_tensor("wvT", [D, F], BF16, kind="ExternalInput")
    woT = nc.dram_tensor("woT", [F, D], BF16, kind="ExternalInput")
    cos_d = nc.dram_tensor("cosT", [128, S], BF16, kind="ExternalInput")
    sin_d = nc.dram_tensor("sinT", [128, S], BF16, kind="ExternalInput")
    msk_d = nc.dram_tensor("masks", [4, 128, 512], BF16, kind="ExternalInput")
    y = nc.dram_tensor("y", [S, D], F32, kind="ExternalOutput")

    xT_r = xT.ap().rearrange("(kt p) s -> p kt s", p=128)       # [128, 16, S]
    wq_r = wqT.ap().rearrange("(kt p) f -> p kt f", p=128)
    wk_r = wkT.ap().rearrange("(kt p) f -> p kt f", p=128)
    wv_r = wvT.ap().rearrange("(kt p) f -> p kt f", p=128)

    with tile.TileContext(nc) as tc, ExitStack() as ctx:
        const = ctx.enter_context(tc.tile_pool(name="const", bufs=1))
        persist = ctx.enter_context(tc.tile_pool(name="persist", bufs=1))
        rope_p = ctx.enter_context(tc.tile_pool(name="rope", bufs=2))
        nrm = ctx.enter_context(tc.tile_pool(name="nrm", bufs=2))
        ystage = ctx.enter_context(tc.tile_pool(name="ystage", bufs=4))
        ps512 = ctx.enter_context(tc.tile_pool(name="ps512", bufs=6, space="PSUM"))
        ps_den = ctx.enter_context(tc.tile_pool(name="ps_den", bufs=2, space="PSUM"))

        ones_f = const.tile([128, 1], F32, tag="ones_f")
        nc.gpsimd.memset(ones_f[:], 1.0)
        ones = const.tile([128, 1], BF16, tag="ones")
        nc.vector.tensor_copy(ones[:], ones_f[:])
        # staging tile for the packed denominator chains: zeroed once; only
        # partitions {0,32,64,96} are ever rewritten, so a plain ones-matmul
        # sums exactly the 4 chains
        d_bf = const.tile([128, 512], BF16, tag="d_bf")
        nc.gpsimd.memset(d_bf[:], 0.0)
        msk_sb = const.tile([128, 4, 512], BF16, tag="masks")
        cos_sb = const.tile([128, S], BF16, tag="cos")
        sin_sb = const.tile([128, S], BF16, tag="sin")
        nc.sync.dma_start(msk_sb[:], msk_d.ap().rearrange("o p q -> p o q"))
        nc.sync.dma_start(cos_sb[:], cos_d.ap())
        nc.sync.dma_start(sin_sb[:], sin_d.ap())

        for _rep in range(reps):
            qT_sb = persist.tile([128, HG, S], BF16, tag="qT", name="qT")
            kT_sb = persist.tile([128, HG, S], BF16, tag="kT", name="kT")
            vh_sb = persist.tile([128, NT, F], BF16, tag="vh", name="vh")
            attn_sb = persist.tile([128, HG, S], BF16, tag="attn", name="attn")

            # ---------- Projections: qT/kT transposed, v natural ----------
            with ExitStack() as actx:
                wpool = actx.enter_context(tc.tile_pool(name="wpool", bufs=1))
                xpool = actx.enter_context(tc.tile_pool(name="xpool", bufs=2))

                wq_sb = wpool.tile([128, NT, F], BF16, tag="wq")
                wk_sb = wpool.tile([128, NT, F], BF16, tag="wk")
                wv_sb = wpool.tile([128, NT, F], BF16, tag="wv")
                x_tiles = {0: xpool.tile([128, NT, 512], BF16, tag="x", name="x0")}
                # interleave per-kt weight/x chunks so the PE starts early
                for kt in range(NT):
                    nc.sync.dma_start(wq_sb[:, kt, :], wq_r[:, kt, :])
                    nc.sync.dma_start(x_tiles[0][:, kt, :], xT_r[:, kt, ts(0, 512)])
                    nc.sync.dma_start(wk_sb[:, kt, :], wk_r[:, kt, :])
                    nc.sync.dma_start(wv_sb[:, kt, :], wv_r[:, kt, :])

                for tb in range(TB):
                    if tb in x_tiles:
                        x_sb = x_tiles[tb]
                    else:
                        x_sb = xpool.tile([128, NT, 512], BF16, tag="x")
                        for kt in range(NT):
                            nc.sync.dma_start(x_sb[:, kt, :],
                                              xT_r[:, kt, ts(tb, 512)])
                    # q and k projections, transposed output [feature, token]
                    for (w_sb, dstT, rtag) in ((wq_sb, qT_sb, "q"),
                                               (wk_sb, kT_sb, "k")):
                        for f in range(HG):
                            ps = ps512.tile([128, 512], F32, tag="ps512")
                            for kt in range(NT):
                                nc.tensor.matmul(ps[:],
                                                 w_sb[:, kt, ts(f, 128)],
                                                 x_sb[:, kt, :],
                                                 start=(kt == 0),
                                                 stop=(kt == NT - 1))
                            # RoPE: rotate_half swaps partition halves.
                            # Tensor-tensor ops need matching input bases, so
                            # stage PSUM->SBUF (ScalarE), swap halves with two
                            # single-input DVE copies (shifted bases are legal
                            # for copies, and same-engine order kills races);
                            # the sin table carries the rotate_half signs.
                            s_q = rope_p.tile([128, 512], BF16, tag=f"sq{rtag}")
                            nc.scalar.copy(s_q[:], ps[:])
                            swp = rope_p.tile([128, 512], BF16, tag=f"sw{rtag}")
                            nc.vector.tensor_copy(swp[0:64, :], s_q[64:128, :])
                            nc.vector.tensor_copy(swp[64:128, :], s_q[0:64, :])
                            tmp = rope_p.tile([128, 512], BF16, tag=f"tmp{rtag}")
                            nc.vector.tensor_mul(tmp[:], swp[:],
                                                 sin_sb[:, ts(tb, 512)])
                            qc = rope_p.tile([128, 512], BF16, tag=f"qc{rtag}")
                            nc.vector.tensor_mul(qc[:], s_q[:],
                                                 cos_sb[:, ts(tb, 512)])
                            nc.vector.tensor_add(dstT[:, f, ts(tb, 512)],
                                                 qc[:], tmp[:])
                    # v projection, natural layout [token, feature]
                    for tt in range(4):
                        pv = ps512.tile([128, 512], F32, tag="ps512")
                        for kt in range(NT):
                            nc.tensor.matmul(pv[:],
                                             x_sb[:, kt, ts(tt, 128)],
                                             wv_sb[:, kt, :],
                                             start=(kt == 0),
                                             stop=(kt == NT - 1))
                        nc.scalar.copy(vh_sb[:, 4 * tb + tt, :], pv[:])

            # ---------- Attention + output projection ----------
            with ExitStack() as bctx:
                wopool = bctx.enter_context(tc.tile_pool(name="wopool", bufs=1))
                pt_pool = bctx.enter_context(tc.tile_pool(name="pt", bufs=8))

                wo_sb = wopool.tile([128, G, D], BF16, tag="wo")
                nc.sync.dma_start(wo_sb[:],
                                  woT.ap().rearrange("(ft p) d -> p ft d", p=128))

                for qb in range(NQB):
                    nkt = 4 * qb + 4
                    for h in range(HG):
                        p_att = ps512.tile([128, 512], F32, tag="ps512")
                        den = ps_den.tile([128, 512], F32, tag="den")
                        for kt in range(nkt):
                            p_s = ps512.tile([128, 512], F32, tag="ps512")
                            nc.tensor.matmul(p_s[:],
                                             kT_sb[:, h, ts(kt, 128)],
                                             qT_sb[:, h, ts(qb, 512)],
                                             start=True, stop=True)
                            pt = pt_pool.tile([128, 512], BF16, tag="pt")
                            nc.scalar.activation(pt[:], p_s[:],
                                                 mybir.ActivationFunctionType.Exp,
                                                 scale=1.0 / math.sqrt(DH))
                            if kt >= 4 * qb:
                                nc.vector.tensor_mul(pt[:], pt[:],
                                                     msk_sb[:, kt - 4 * qb, :])
                            nc.tensor.matmul(p_att[:],
                                             vh_sb[:, kt, ts(h, 128)],
                                             pt[:],
                                             start=(kt == 0),
                                             stop=(kt == nkt - 1))
                            j = kt % 4
                            nc.tensor.matmul(den[32 * j:32 * j + 1, :],
                                             ones[:], pt[:],
                                             start=(kt < 4),
                                             stop=(kt >= nkt - 4),
                                             tile_position=(0, 32 * j))
                        # den total: stage the 4 packed chains to SBUF
                        # (aligned single-partition copies), sum via a rank-1
                        # ones-matmul reusing the den bank for the result
                        for j in range(4):
                            nc.scalar.copy(d_bf[32 * j:32 * j + 1, :],
                                           den[32 * j:32 * j + 1, :])
                        nc.tensor.matmul(den[0:1, :], ones[:], d_bf[:],
                                         start=True, stop=True)
                        recip = nrm.tile([1, 512], F32, tag="recip")
                        nc.vector.reciprocal_approx_fast(recip[:], den[0:1, :])
                        rb = nrm.tile([128, 512], F32, tag="rb")
                        nc.gpsimd.partition_broadcast(rb[:], recip[:])
                        nc.vector.tensor_mul(attn_sb[:, h, ts(qb, 512)],
                                             p_att[:], rb[:])
                    # output projection for this query block
                    for qt in range(4 * qb, 4 * qb + 4):
                        for ddb in range(NQB):
                            py = ps512.tile([128, 512], F32, tag="ps512")
                            for ft in range(G):
                                nc.tensor.matmul(py[:],
                                                 attn_sb[:, ft, ts(qt, 128)],
                                                 wo_sb[:, ft, ts(ddb, 512)],
                                                 start=(ft == 0),
                                                 stop=(ft == G - 1))
                            y_sb = ystage.tile([128, 512], F32, tag="ysb")
                            nc.scalar.copy(y_sb[:], py[:])
                            nc.sync.dma_start(y.ap()[ts(qt, 128), ts(ddb, 512)],
                                              y_sb[:])

    nc.compile()
    _cache[key] = nc
    return nc


def _in_maps(hidden_q, Wq, Wk, Wv, Wo):
    xs = (hidden_q.astype(np.float32) / math.sqrt(D))
    xT = [np.ascontiguousarray(xs[b].T).astype(NPBF) for b in range(B)]
    cos_full, sin_signed = _rope_tables()
    masks = _mask_tiles()
    wo_s = Wo.astype(np.float32) / math.sqrt(H * DH)
    in_maps = []
    for c in range(8):
        b, g = c // G, c % G
        rows = slice(F * g, F * (g + 1))
        in_maps.append({
            "xT": xT[b],
            "wqT": np.ascontiguousarray(Wq[rows, :].T).astype(NPBF),
            "wkT": np.ascontiguousarray(Wk[rows, :].T).astype(NPBF),
            "wvT": np.ascontiguousarray(Wv[rows, :].T).astype(NPBF),
            "woT": np.ascontiguousarray(wo_s[:, rows].T).astype(NPBF),
            "cosT": cos_full, "sinT": sin_signed, "masks": masks,
        })
    return in_maps


def kernel(hidden_q, attention_mask, position_bias, Wq, Wk, Wv, Wo):
    hidden_q = np.asarray(hidden_q)
    Wq, Wk, Wv, Wo = (np.asarray(w) for w in (Wq, Wk, Wv, Wo))
    assert hidden_q.shape == (B, S, D)
    in_maps = _in_maps(hidden_q, Wq, Wk, Wv, Wo)
    nc = _build()
    res = run_bass_kernel_spmd(nc, in_maps, core_ids=list(range(8)))
    _cache["last_results"] = res
    out = np.zeros((B, S, D), np.float32)
    for c in range(8):
        out[c // G] += res.results[c]["y"]
    return out


# revision 22
# speedup vs baseline: 1.0586x; 1.0245x over previous
"""Multi-head causal self-attention with RoPE on 8 Trainium2 NeuronCores.

Reference computation (B=2, S=2048, D=2048, H=16, DH=128):
    xs = hidden_q / sqrt(D)
    q,k,v = xs @ {Wq,Wk,Wv}.T        (reshaped to [B,H,S,DH])
    q,k <- RoPE(q,k)
    scores = q @ k.T / sqrt(DH)  (causal masked)
    p = softmax(scores); attn = p @ v
    out = (attn / sqrt(H*DH)) @ Wo.T

Sharding: 8 cores = 2 (batch) x 4 (head-groups of 4 heads).  Each core
computes its head-group's projections, attention and a partial output
projection; the host sums the 4 partials per batch.

v2 design (vs v1): all matmul operands in bf16 (PSUM accumulation stays
fp32); q/k are produced DIRECTLY TRANSPOSED by swapping matmul roles
(weights stationary, x moving -> out[feature, token]), which removes all
128 PE transposes, the 8MB q/k DRAM spill roundtrip, and the phase
barrier.  RoPE runs on DVE with partition-shifted APs (rotate_half swaps
partition halves in the transposed layout; the sin table carries the
sign).  Softmax denominators (ones-matmuls, M=1) are packed 4-at-a-time
onto disjoint PE column groups via tile_position so they run
concurrently (~4x less PE time than v1's serial den matmuls).

Layouts on device (per core):
    xT    [D, S]    feature-major activations (host pre-transposed/scaled)
    wqT   [D, 512]  per-group Wq slice, transposed (ditto wk, wv)
    woT   [512, D]  per-group Wo slice, transposed, prescaled
    qT/kT [128, h, S] SBUF: partition=dh-within-head, free=[head, token]
    vh    [128, tile, 512] SBUF: natural [token, feature] tiles
    scoresT [keys, queries]; denominators via packed ones-matmuls
    y     [S, D]    natural-layout partial output (host sums over groups)
"""

import math
from contextlib import ExitStack

import ml_dtypes
import numpy as np

import concourse.bass as bass
import concourse.mybir as mybir
import concourse.tile as tile
from concourse import bacc
from concourse.bass import ts
from concourse.bass_utils import run_bass_kernel_spmd

B, S, D, H, DH = 2, 2048, 2048, 16, 128
BASE = 10000.0
G = 4              # head-groups (cores per batch)
HG = H // G        # heads per group = 4
F = HG * DH        # features per group = 512
NT = S // 128      # 16 token tiles of 128
TB = S // 512      # 4 token blocks of 512
NQB = S // 512     # 4 query blocks
F32 = mybir.dt.float32
BF16 = mybir.dt.bfloat16
NPBF = ml_dtypes.bfloat16

_cache = {}


def _rope_tables():
    inv_freq = 1.0 / (BASE ** (np.arange(0, DH, 2, dtype=np.float64) / DH))
    t = np.arange(S, dtype=np.float64)
    freqs = np.outer(t, inv_freq)                       # [S, 64]
    cosT = np.cos(freqs).T.astype(np.float32)           # [64, S]
    sinT = np.sin(freqs).T.astype(np.float32)
    cos_full = np.ascontiguousarray(np.concatenate([cosT, cosT], 0)).astype(NPBF)
    sin_signed = np.ascontiguousarray(np.concatenate([-sinT, sinT], 0)).astype(NPBF)
    return cos_full, sin_signed                         # [128, S] each, bf16


def _mask_tiles():
    # mask[o][j, q] = 1 if q >= j + 128*o else 0  (diagonal-band tiles)
    o = np.arange(4)[:, None, None]
    j = np.arange(128)[None, :, None]
    q = np.arange(512)[None, None, :]
    return (q >= j + 128 * o).astype(NPBF)              # [4, 128, 512]


def _build(reps=1):
    key = ("nc", reps)
    if key in _cache:
        return _cache[key]
    nc = bacc.Bacc("TRN2", target_bir_lowering=False, debug=False, num_devices=8)

    xT = nc.dram_tensor("xT", [D, S], BF16, kind="ExternalInput")
    wqT = nc.dram_tensor("wqT", [D, F], BF16, kind="ExternalInput")
    wkT = nc.dram_tensor("wkT", [D, F], BF16, kind="ExternalInput")
    wvT = nc.dram_tensor("wvT", [D, F], BF16, kind="ExternalInput")
    woT = nc.dram_tensor("woT", [F, D], BF16, kind="ExternalInput")
    cos_d = nc.dram_tensor("cosT", [128, S], BF16, kind="ExternalInput")
    sin_d = nc.dram_tensor("sinT", [128, S], BF16, kind="ExternalInput")
    msk_d = nc.dram_tensor("masks", [4, 128, 512], BF16, kind="ExternalInput")
    y = nc.dram_tensor("y", [S, D], F32, kind="ExternalOutput")

    xT_r = xT.ap().rearrange("(kt p) s -> p kt s", p=128)       # [128, 16, S]
    wq_r = wqT.ap().rearrange("(kt p) f -> p kt f", p=128)
    wk_r = wkT.ap().rearrange("(kt p) f -> p kt f", p=128)
    wv_r = wvT.ap().rearrange("(kt p) f -> p kt f", p=128)

    with tile.TileContext(nc) as tc, ExitStack() as ctx:
        const = ctx.enter_context(tc.tile_pool(name="const", bufs=1))
        persist = ctx.enter_context(tc.tile_pool(name="persist", bufs=1))
        rope_p = ctx.enter_context(tc.tile_pool(name="rope", bufs=2))
        nrm = ctx.enter_context(tc.tile_pool(name="nrm", bufs=2))
        ystage = ctx.enter_context(tc.tile_pool(name="ystage", bufs=3))
        xpool = ctx.enter_context(tc.tile_pool(name="xpool", bufs=2))
        pt_pool = ctx.enter_context(tc.tile_pool(name="pt", bufs=6))
        ps512 = ctx.enter_context(tc.tile_pool(name="ps512", bufs=4, space="PSUM"))
        ps_att = ctx.enter_context(tc.tile_pool(name="ps_att", bufs=2, space="PSUM"))
        ps_den = ctx.enter_context(tc.tile_pool(name="ps_den", bufs=2, space="PSUM"))

        ones_f = const.tile([128, 1], F32, tag="ones_f")
        nc.gpsimd.memset(ones_f[:], 1.0)
        ones = const.tile([128, 1], BF16, tag="ones")
        nc.vector.tensor_copy(ones[:], ones_f[:])
        # staging tile for the packed denominator chains: zeroed once; only
        # partitions {0,32,64,96} are ever rewritten, so a plain ones-matmul
        # sums exactly the 4 chains
        d_bf = const.tile([128, 512], BF16, tag="d_bf")
        nc.gpsimd.memset(d_bf[:], 0.0)
        msk_sb = const.tile([128, 4, 512], BF16, tag="masks")
        cos_sb = const.tile([128, S], BF16, tag="cos")
        sin_sb = const.tile([128, S], BF16, tag="sin")
        nc.sync.dma_start(msk_sb[:], msk_d.ap().rearrange("o p q -> p o q"))
        nc.sync.dma_start(cos_sb[:], cos_d.ap())
        nc.sync.dma_start(sin_sb[:], sin_d.ap())

        for _rep in range(reps):
            qT_sb = persist.tile([128, HG, S], BF16, tag="qT", name="qT")
            kT_sb = persist.tile([128, HG, S], BF16, tag="kT", name="kT")
            vh_sb = persist.tile([128, NT, F], BF16, tag="vh", name="vh")
            attn_sb = persist.tile([128, HG, S], BF16, tag="attn", name="attn")
            wq_sb = persist.tile([128, NT, F], BF16, tag="wq", name="wq")
            wk_sb = persist.tile([128, NT, F], BF16, tag="wk", name="wk")
            wv_sb = persist.tile([128, NT, F], BF16, tag="wv", name="wv")
            wo_sb = persist.tile([128, G, D], BF16, tag="wo", name="wo")

            x_tiles = {0: xpool.tile([128, NT, 512], BF16, tag="x", name="x0")}
            # interleave per-kt weight/x chunks so the PE starts early
            for kt in range(NT):
                nc.sync.dma_start(wq_sb[:, kt, :], wq_r[:, kt, :])
                nc.sync.dma_start(x_tiles[0][:, kt, :], xT_r[:, kt, ts(0, 512)])
                nc.sync.dma_start(wk_sb[:, kt, :], wk_r[:, kt, :])
                nc.sync.dma_start(wv_sb[:, kt, :], wv_r[:, kt, :])
            nc.sync.dma_start(wo_sb[:],
                              woT.ap().rearrange("(ft p) d -> p ft d", p=128))

            def proj(tb):
                """q/k (transposed + RoPE) and v projections for token
                block tb."""
                if tb in x_tiles:
                    x_sb = x_tiles[tb]
                else:
                    x_sb = xpool.tile([128, NT, 512], BF16, tag="x")
                    for kt in range(NT):
                        nc.sync.dma_start(x_sb[:, kt, :],
                                          xT_r[:, kt, ts(tb, 512)])
                for (w_sb, dstT, rtag) in ((wq_sb, qT_sb, "q"),
                                           (wk_sb, kT_sb, "k")):
                    for f in range(HG):
                        ps = ps512.tile([128, 512], F32, tag="ps512")
                        for kt in range(NT):
                            nc.tensor.matmul(ps[:],
                                             w_sb[:, kt, ts(f, 128)],
                                             x_sb[:, kt, :],
                                             start=(kt == 0),
                                             stop=(kt == NT - 1))
                        # RoPE: rotate_half swaps partition halves.
                        # Tensor-tensor ops need matching input bases, so
                        # stage PSUM->SBUF (ScalarE), swap halves with two
                        # single-input DVE copies (shifted bases are legal
                        # for copies, same-engine order kills races); the
                        # sin table carries the rotate_half signs.
                        s_q = rope_p.tile([128, 512], BF16, tag="sq")
                        nc.scalar.copy(s_q[:], ps[:])
                        swp = rope_p.tile([128, 512], BF16, tag="sw")
                        nc.vector.tensor_copy(swp[0:64, :], s_q[64:128, :])
                        nc.vector.tensor_copy(swp[64:128, :], s_q[0:64, :])
                        tmp = rope_p.tile([128, 512], BF16, tag="tmp")
                        nc.vector.tensor_mul(tmp[:], swp[:],
                                             sin_sb[:, ts(tb, 512)])
                        qc = rope_p.tile([128, 512], BF16, tag="qc")
                        nc.vector.tensor_mul(qc[:], s_q[:],
                                             cos_sb[:, ts(tb, 512)])
                        nc.vector.tensor_add(dstT[:, f, ts(tb, 512)],
                                             qc[:], tmp[:])
                # v projection, natural layout [token, feature]
                for tt in range(4):
                    pv = ps512.tile([128, 512], F32, tag="ps512")
                    for kt in range(NT):
                        nc.tensor.matmul(pv[:],
                                         x_sb[:, kt, ts(tt, 128)],
                                         wv_sb[:, kt, :],
                                         start=(kt == 0),
                                         stop=(kt == NT - 1))
                    nc.scalar.copy(vh_sb[:, 4 * tb + tt, :], pv[:])

            def attention(qb):
                """Scores/softmax/AV + output projection for query block
                qb (kt in groups of 4 so the packed denominator matmuls sit
                back-to-back on disjoint PE column groups)."""
                nkt = 4 * qb + 4
                for h in range(HG):
                    p_att = ps_att.tile([128, 512], F32, tag="p_att")
                    den = ps_den.tile([128, 512], F32, tag="den")
                    for g in range(nkt // 4):
                        pts = []
                        for kt in range(4 * g, 4 * g + 4):
                            p_s = ps512.tile([128, 512], F32, tag="ps512")
                            nc.tensor.matmul(p_s[:],
                                             kT_sb[:, h, ts(kt, 128)],
                                             qT_sb[:, h, ts(qb, 512)],
                                             start=True, stop=True)
                            pt = pt_pool.tile([128, 512], BF16, tag="pt")
                            nc.scalar.activation(pt[:], p_s[:],
                                                 mybir.ActivationFunctionType.Exp,
                                                 scale=1.0 / math.sqrt(DH))
                            if kt >= 4 * qb:
                                nc.vector.tensor_mul(pt[:], pt[:],
                                                     msk_sb[:, kt - 4 * qb, :])
                            nc.tensor.matmul(p_att[:],
                                             vh_sb[:, kt, ts(h, 128)],
                                             pt[:],
                                             start=(kt == 0),
                                             stop=(kt == nkt - 1))
                            pts.append(pt)
                        for j, pt in enumerate(pts):
                            nc.tensor.matmul(den[32 * j:32 * j + 1, :],
                                             ones[:], pt[:],
                                             start=(g == 0),
                                             stop=(g == nkt // 4 - 1),
                                             tile_position=(0, 32 * j))
                    # den total: stage the 4 packed chains to SBUF (aligned
                    # single-partition copies), sum via a rank-1 ones-matmul
                    # reusing the den bank for the result
                    for j in range(4):
                        nc.vector.tensor_copy(d_bf[32 * j:32 * j + 1, :],
                                              den[32 * j:32 * j + 1, :])
                    nc.tensor.matmul(den[0:1, :], ones[:], d_bf[:],
                                     start=True, stop=True)
                    recip = nrm.tile([1, 512], F32, tag="recip")
                    nc.vector.reciprocal_approx_fast(recip[:], den[0:1, :])
                    rb = nrm.tile([128, 512], F32, tag="rb")
                    nc.gpsimd.partition_broadcast(rb[:], recip[:])
                    nc.vector.tensor_mul(attn_sb[:, h, ts(qb, 512)],
                                         p_att[:], rb[:])
                # output projection for this query block
                for qt in range(4 * qb, 4 * qb + 4):
                    for ddb in range(NQB):
                        py = ps512.tile([128, 512], F32, tag="ps512")
                        for ft in range(G):
                            nc.tensor.matmul(py[:],
                                             attn_sb[:, ft, ts(qt, 128)],
                                             wo_sb[:, ft, ts(ddb, 512)],
                                             start=(ft == 0),
                                             stop=(ft == G - 1))
                        y_sb = ystage.tile([128, 512], F32, tag="ysb")
                        nc.vector.tensor_copy(y_sb[:], py[:])
                        nc.sync.dma_start(y.ap()[ts(qt, 128), ts(ddb, 512)],
                                          y_sb[:])

            # Software pipeline: attention(qb) is emitted right after
            # proj(qb) — the scheduler fills attention's exp-wait PE stalls
            # with proj(qb+1) matmuls while program order keeps attention at
            # higher priority, so the ScalarE exp cost hides under
            # projection matmuls.
            for tb in range(TB):
                proj(tb)
                attention(tb)

    nc.compile()
    _cache[key] = nc
    return nc


def _in_maps(hidden_q, Wq, Wk, Wv, Wo):
    xs = (hidden_q.astype(np.float32) / math.sqrt(D))
    xT = [np.ascontiguousarray(xs[b].T).astype(NPBF) for b in range(B)]
    cos_full, sin_signed = _rope_tables()
    masks = _mask_tiles()
    wo_s = Wo.astype(np.float32) / math.sqrt(H * DH)
    in_maps = []
    for c in range(8):
        b, g = c // G, c % G
        rows = slice(F * g, F * (g + 1))
        in_maps.append({
            "xT": xT[b],
            "wqT": np.ascontiguousarray(Wq[rows, :].T).astype(NPBF),
            "wkT": np.ascontiguousarray(Wk[rows, :].T).astype(NPBF),
            "wvT": np.ascontiguousarray(Wv[rows, :].T).astype(NPBF),
            "woT": np.ascontiguousarray(wo_s[:, rows].T).astype(NPBF),
            "cosT": cos_full, "sinT": sin_signed, "masks": masks,
        })
    return in_maps


def kernel(hidden_q, attention_mask, position_bias, Wq, Wk, Wv, Wo):
    hidden_q = np.asarray(hidden_q)
    Wq, Wk, Wv, Wo = (np.asarray(w) for w in (Wq, Wk, Wv, Wo))
    assert hidden_q.shape == (B, S, D)
    in_maps = _in_maps(hidden_q, Wq, Wk, Wv, Wo)
    nc = _build()
    res = run_bass_kernel_spmd(nc, in_maps, core_ids=list(range(8)))
    _cache["last_results"] = res
    out = np.zeros((B, S, D), np.float32)
    for c in range(8):
        out[c // G] += res.results[c]["y"]
    return out
